# revision 6
# baseline (speedup 1.0000x reference)
"""Fused ACNet-style 5-branch conv block as a single 3x3 conv on Trainium2.

The reference computes
    out = conv3x3(x, w_square) + conv3x1(x, w_ver) + conv1x3(x, w_hor)
        + conv3x3(x, w_diag19 * eye3) + conv3x3(x, w_diag37 * antieye3)
All five branches are linear convs with identical output geometry, so they
fold into ONE effective 3x3 conv whose weight is the sum of the embedded /
masked branch weights.  The conv runs as 9 shifted matmuls (one per tap)
accumulated in PSUM, channels on the 128 SBUF partitions (C_in = C_out = 128).

Layout (v3): x rows are packed at stride 208 elements (192 px + 1-px border
on each side + zero fill), so every matmul's moving operand starts exactly at
a row boundary — 416-byte (32B-aligned) strides.  The per-tap kw shift is
applied on the PSUM WRITE side instead of the SBUF read side:
    stream n of packed row (r+kh) lands at psum col n + (2-kw),
    so out pixel w always sits at psum col w+2, for every tap.
Misaligned moving-operand streams measurably throttle the PE stream rate
(~12% on this kernel), so keeping every stream row-aligned matters more than
the ~1% extra columns this costs.

Everything runs in bf16 (x, w, y), accumulating in fp32 PSUM: bf16 streams
one moving column per PE cycle at full clock (2x the fp32 byte-limited rate)
and halves the HBM traffic.  Loop order is taps-OUTER over a group of 8 PSUM
banks (16 output rows): one weight load per tap per group instead of one per
matmul.  Each bank's drain copy is issued right after that bank's final-tap
matmul so drains overlap the remaining banks' matmuls and the next group
never stalls on a busy PSUM bank.  The first tap processed is (kh=0, kw=2)
with a widened 198-col stream so its start=True write initializes the whole
psum window.

Sharding: data-parallel over batch — 16 images / 8 cores = 2 images per
core, weights replicated, no collectives.
"""

import sys

for _p in ("/opt/trn_rl_repo",):
    if _p not in sys.path:
        sys.path.insert(0, _p)

import numpy as np

import concourse.mybir as mybir
import concourse.tile as tile
from concourse import bacc
from concourse.bass_utils import run_bass_kernel_spmd

B, C, H, W = 16, 128, 192, 192
NCORES = 8
IPC = B // NCORES  # images per core
NTAP = 9
RS = 208  # packed row stride (192 px + 2 border + 14 zero fill, 32B-aligned)
NROWS = H + 2  # packed rows per image (1-px border top/bottom)
PW = 198  # psum window width per output row (2 + 192 + 4 slack)
RB = 32  # output rows per x-load block
GB = 8  # PSUM banks per matmul group (8 output rows, 1 row per bank)
# (kh, kw, stream_len): first tap streams 198 cols so start=True initializes
# the full psum window; the rest stream just enough to cover out cols 2..195.
TAPS = [(0, 2, 198), (0, 0, 194), (0, 1, 195), (1, 0, 194), (1, 1, 195),
        (1, 2, 196), (2, 0, 194), (2, 1, 195), (2, 2, 196)]
MM_DT = mybir.dt.bfloat16
NP_BF16 = mybir.dt.np(mybir.dt.bfloat16)


def _build(ipc, rb, mm_dt, repeat=1, xbufs=3, obufs=2, ahead=1):
    """Emit the per-core Bass program.

    The x-DMA for block k+ahead is issued before block k's compute/out-DMA
    in program order, so input prefetch never queues behind output drains.
    repeat>1 wraps the body in a For_i loop (timing harness only; the body
    is idempotent so outputs are unchanged).
    """
    nc = bacc.Bacc("TRN2", target_bir_lowering=False, debug=False)
    x_in = nc.dram_tensor(
        "x", [ipc, C, NROWS, RS], mm_dt, kind="ExternalInput"
    ).ap()
    w_in = nc.dram_tensor(
        "w", [C, NTAP * C], mm_dt, kind="ExternalInput"
    ).ap()
    y_out = nc.dram_tensor(
        "y", [ipc, C, H, W], mm_dt, kind="ExternalOutput"
    ).ap()

    xrows = rb + 2  # packed rows per x tile
    blocks = [(img, r0) for img in range(ipc) for r0 in range(0, H, rb)]
    rpg = GB  # output rows per matmul group (1 row per PSUM bank)

    with tile.TileContext(nc) as tc:
        with (
            tc.tile_pool(name="wp", bufs=1) as wpool,
            tc.tile_pool(name="xp", bufs=xbufs) as xpool,
            tc.tile_pool(name="op", bufs=obufs) as opool,
            tc.tile_pool(name="ps", bufs=8, space="PSUM") as pspool,
        ):
            wt = wpool.tile([C, NTAP * C], mm_dt)
            nc.sync.dma_start(wt[:], w_in[:])

            def load(img, r0):
                xt = xpool.tile([C, xrows, RS], mm_dt, tag="xt",
                                name=f"xt{img}_{r0}")
                nc.sync.dma_start(xt[:], x_in[img, :, r0 : r0 + xrows, :])
                return xt

            def body():
                xts = [load(*blocks[k]) for k in range(min(ahead, len(blocks)))]
                for k, (img, r0) in enumerate(blocks):
                    if k + ahead < len(blocks):
                        xts.append(load(*blocks[k + ahead]))
                    xt = xts.pop(0)
                    ot = opool.tile([C, rb, W], mm_dt, tag="ot",
                                    name=f"ot{img}_{r0}")
                    for g in range(rb // rpg):
                        pss = [
                            pspool.tile([C, PW], mybir.dt.float32,
                                        tag="ps", name=f"ps{p}")
                            for p in range(GB)
                        ]
                        for t, (kh, kw, sl) in enumerate(TAPS):
                            wc = (kh * 3 + kw) * C
                            co = 2 - kw  # psum col where stream col 0 lands
                            for p in range(GB):
                                lr = g * rpg + p
                                nc.tensor.matmul(
                                    pss[p][:, co : co + sl],
                                    wt[:, wc : wc + C],
                                    xt[:, lr + kh, 0:sl],
                                    start=(t == 0),
                                    stop=(t == NTAP - 1),
                                )
                                if t == NTAP - 1:
                                    eng = (nc.scalar.copy if p % 2 == 0
                                           else nc.vector.tensor_copy)
                                    eng(ot[:, lr, :],
                                        pss[p][:, 2 : 2 + W])
                    nc.sync.dma_start(y_out[img, :, r0 : r0 + rb, :], ot[:])

            if repeat == 1:
                body()
            else:
                with tc.For_i(0, repeat, 1):
                    body()
    nc.compile()
    return nc


def _fold_weights(w_square, w_ver, w_hor, w_diag19, w_diag37):
    """Fold the 5 branches into one 3x3 weight, laid out [C_in, tap*C_out]."""
    eye = np.eye(3, dtype=np.float32)
    anti = eye[::-1, :]
    w_eff = (
        np.asarray(w_square, np.float32)
        + np.asarray(w_diag19, np.float32) * eye
        + np.asarray(w_diag37, np.float32) * anti
    )
    w_eff[:, :, :, 1] += np.asarray(w_ver, np.float32)[:, :, :, 0]
    w_eff[:, :, 1, :] += np.asarray(w_hor, np.float32)[:, :, 0, :]
    # [O, I, KH, KW] -> [I, KH, KW, O] -> [I, (KH*KW)*O]  (lhsT per tap)
    return np.ascontiguousarray(
        w_eff.transpose(1, 2, 3, 0).reshape(C, NTAP * C)
    ).astype(NP_BF16)


def _pack_x(x):
    """[B,C,H,W] -> row-aligned packed bf16 [B,C,NROWS,RS].

    packed[:, :, i, n] = x_pad[i-1, n-1]: 1-px zero border on all sides,
    rows padded to RS elements with zeros.
    """
    xs = np.zeros((B, C, NROWS, RS), NP_BF16)
    xs[:, :, 1 : H + 1, 1 : W + 1] = np.asarray(x, np.float32).astype(NP_BF16)
    return xs


_nc_cache = {}


def kernel(x, w_square, w_ver, w_hor, w_diag19, w_diag37):
    w_host = _fold_weights(w_square, w_ver, w_hor, w_diag19, w_diag37)
    xs = _pack_x(x)

    if "nc" not in _nc_cache:
        _nc_cache["nc"] = _build(IPC, RB, MM_DT)
    nc = _nc_cache["nc"]

    in_maps = [
        {"x": np.ascontiguousarray(xs[c * IPC : (c + 1) * IPC]), "w": w_host}
        for c in range(NCORES)
    ]
    res = run_bass_kernel_spmd(nc, in_maps, list(range(NCORES)))
    return np.concatenate(
        [res.results[c]["y"] for c in range(NCORES)], axis=0
    ).astype(np.float32)


# revision 7
# speedup vs baseline: 1.0069x; 1.0069x over previous
"""Fused ACNet-style 5-branch conv block as a single 3x3 conv on Trainium2.

The reference computes
    out = conv3x3(x, w_square) + conv3x1(x, w_ver) + conv1x3(x, w_hor)
        + conv3x3(x, w_diag19 * eye3) + conv3x3(x, w_diag37 * antieye3)
All five branches are linear convs with identical output geometry, so they
fold into ONE effective 3x3 conv whose weight is the sum of the embedded /
masked branch weights.  The conv runs as 9 shifted matmuls (one per tap)
accumulated in PSUM, channels on the 128 SBUF partitions (C_in = C_out = 128):
    out[:, h, w] += W[kh,kw].T @ x_pad[:, h+kh, w+kw]

Input layout: spacer-packed rows — each padded row is 193 elements (192 data
+ 1 shared zero spacer).  The spacer acts as right-pad of row r AND left-pad
of row r+1, so every tap shift is a pure flat offset and each matmul's moving
operand is ONE contiguous 386-element run (2 output rows per PSUM bank).

Everything runs in bf16 (x, w, y), accumulating in fp32 PSUM: bf16 streams
one moving column per 2.4 GHz PE cycle (2x the fp32 byte-limited rate) and
halves the HBM traffic.  Loop order is taps-OUTER over a group of 8 PSUM
banks (16 output rows): one weight load per tap per group (amortized over 8
matmuls) instead of one per matmul.  Each bank's drain copy is issued right
after that bank's final-tap matmul so drains overlap the remaining banks'
matmuls and the next group never stalls on a busy PSUM bank.

Sharding: data-parallel over batch — 16 images / 8 cores = 2 images per
core, weights replicated, no collectives.
"""

import sys

for _p in ("/opt/trn_rl_repo",):
    if _p not in sys.path:
        sys.path.insert(0, _p)

import numpy as np

import concourse.mybir as mybir
import concourse.tile as tile
from concourse import bacc
from concourse.bass_utils import run_bass_kernel_spmd

B, C, H, W = 16, 128, 192, 192
NCORES = 8
IPC = B // NCORES  # images per core
NTAP = 9
SW = W + 1  # spacer-packed row width (193)
XLEN = 1 + (H + 2) * SW + 4  # leading zero + 194 packed rows + tap margin
RB = 48  # output rows per x-load block
GB = 8  # PSUM banks per matmul group (16 output rows)
MM_DT = mybir.dt.bfloat16
NP_BF16 = mybir.dt.np(mybir.dt.bfloat16)


def _build(ipc, rb, mm_dt, repeat=1, xbufs=3, obufs=2, ahead=2):
    """Emit the per-core Bass program.

    The x-DMA for block k+ahead is issued before block k's compute/out-DMA
    in program order, so input prefetch never queues behind output drains.
    repeat>1 wraps the body in a For_i loop (timing harness only; the body
    is idempotent so outputs are unchanged).
    """
    nc = bacc.Bacc("TRN2", target_bir_lowering=False, debug=False)
    x_in = nc.dram_tensor(
        "x", [ipc, C, XLEN], mm_dt, kind="ExternalInput"
    ).ap()
    w_in = nc.dram_tensor(
        "w", [C, NTAP * C], mm_dt, kind="ExternalInput"
    ).ap()
    y_out = nc.dram_tensor(
        "y", [ipc, C, H, W], mm_dt, kind="ExternalOutput"
    ).ap()

    xtl = (rb + 2) * SW + 4  # x tile flat length per partition
    blocks = [(img, r0) for img in range(ipc) for r0 in range(0, H, rb)]
    rows_per_group = 2 * GB

    with tile.TileContext(nc) as tc:
        with (
            tc.tile_pool(name="wp", bufs=1) as wpool,
            tc.tile_pool(name="xp", bufs=xbufs) as xpool,
            tc.tile_pool(name="op", bufs=obufs) as opool,
            tc.tile_pool(name="ps", bufs=8, space="PSUM") as pspool,
        ):
            wt = wpool.tile([C, NTAP * C], mm_dt)
            nc.sync.dma_start(wt[:], w_in[:])

            def load(img, r0):
                xt = xpool.tile([C, xtl], mm_dt, tag="xt", name=f"xt{img}_{r0}")
                base = r0 * SW
                nc.sync.dma_start(xt[:], x_in[img, :, base : base + xtl])
                return xt

            def body():
                xts = [load(*blocks[k]) for k in range(min(ahead, len(blocks)))]
                for k, (img, r0) in enumerate(blocks):
                    if k + ahead < len(blocks):
                        xts.append(load(*blocks[k + ahead]))
                    xt = xts.pop(0)
                    # output tile keeps the spacer column; the out-DMA below
                    # reads a [.., 0:W] slice to strip it.
                    ot = opool.tile([C, rb, SW], mm_dt, tag="ot",
                                    name=f"ot{img}_{r0}")
                    for g in range(rb // rows_per_group):
                        pss = [
                            pspool.tile([C, 2 * SW], mybir.dt.float32,
                                        tag="ps", name=f"ps{p}")
                            for p in range(GB)
                        ]
                        for t in range(NTAP):
                            kh, kw = divmod(t, 3)
                            for p in range(GB):
                                row = g * rows_per_group + 2 * p
                                off = (row + kh) * SW + kw
                                nc.tensor.matmul(
                                    pss[p][:],
                                    wt[:, t * C : (t + 1) * C],
                                    xt[:, off : off + 2 * SW],
                                    start=(t == 0),
                                    stop=(t == NTAP - 1),
                                )
                                if t == NTAP - 1:
                                    row = g * rows_per_group + 2 * p
                                    eng = (nc.scalar.copy if p % 2 == 0
                                           else nc.vector.tensor_copy)
                                    eng(ot[:, row : row + 2, :].opt(),
                                        pss[p][:])
                    nc.sync.dma_start(
                        y_out[img, :, r0 : r0 + rb, :], ot[:, :, 0:W]
                    )

            if repeat == 1:
                body()
            else:
                with tc.For_i(0, repeat, 1):
                    body()
    nc.compile()
    return nc


def _fold_weights(w_square, w_ver, w_hor, w_diag19, w_diag37):
    """Fold the 5 branches into one 3x3 weight, laid out [C_in, tap*C_out]."""
    eye = np.eye(3, dtype=np.float32)
    anti = eye[::-1, :]
    w_eff = (
        np.asarray(w_square, np.float32)
        + np.asarray(w_diag19, np.float32) * eye
        + np.asarray(w_diag37, np.float32) * anti
    )
    w_eff[:, :, :, 1] += np.asarray(w_ver, np.float32)[:, :, :, 0]
    w_eff[:, :, 1, :] += np.asarray(w_hor, np.float32)[:, :, 0, :]
    # [O, I, KH, KW] -> [I, KH, KW, O] -> [I, (KH*KW)*O]  (lhsT per tap)
    return np.ascontiguousarray(
        w_eff.transpose(1, 2, 3, 0).reshape(C, NTAP * C)
    ).astype(NP_BF16)


def _pack_x(x):
    """[B,C,H,W] -> spacer-packed flat bf16 [B,C,XLEN]."""
    xs = np.zeros((B, C, XLEN), NP_BF16)
    rows = xs[:, :, 1 : 1 + (H + 2) * SW].reshape(B, C, H + 2, SW)
    rows[:, :, 1 : H + 1, 0:W] = np.asarray(x, np.float32).astype(NP_BF16)
    return xs


_nc_cache = {}


def kernel(x, w_square, w_ver, w_hor, w_diag19, w_diag37):
    w_host = _fold_weights(w_square, w_ver, w_hor, w_diag19, w_diag37)
    xs = _pack_x(x)

    if "nc" not in _nc_cache:
        _nc_cache["nc"] = _build(IPC, RB, MM_DT)
    nc = _nc_cache["nc"]

    in_maps = [
        {"x": np.ascontiguousarray(xs[c * IPC : (c + 1) * IPC]), "w": w_host}
        for c in range(NCORES)
    ]
    res = run_bass_kernel_spmd(nc, in_maps, list(range(NCORES)))
    return np.concatenate(
        [res.results[c]["y"] for c in range(NCORES)], axis=0
    ).astype(np.float32)


# revision 9
# speedup vs baseline: 1.0243x; 1.0174x over previous
"""Fused ACNet-style 5-branch conv block as a single 3x3 conv on Trainium2.

The reference computes
    out = conv3x3(x, w_square) + conv3x1(x, w_ver) + conv1x3(x, w_hor)
        + conv3x3(x, w_diag19 * eye3) + conv3x3(x, w_diag37 * antieye3)
All five branches are linear convs with identical output geometry, so they
fold into ONE effective 3x3 conv whose weight is the sum of the embedded /
masked branch weights.  The conv runs as 9 shifted matmuls (one per tap)
accumulated in PSUM, channels on the 128 SBUF partitions (C_in = C_out = 128):
    out[:, h, w] += W[kh,kw].T @ x_pad[:, h+kh, w+kw]

Input layout: spacer-packed rows — each padded row is 193 elements (192 data
+ 1 shared zero spacer).  The spacer acts as right-pad of row r AND left-pad
of row r+1, so every tap shift is a pure flat offset and each matmul's moving
operand is ONE contiguous 386-element run (2 output rows per PSUM bank).

Everything runs in bf16 (x, w, y), accumulating in fp32 PSUM: bf16 streams
one moving column per 2.4 GHz PE cycle (2x the fp32 byte-limited rate) and
halves the HBM traffic.  Loop order is taps-OUTER over a group of 8 PSUM
banks (16 output rows): one weight load per tap per group (amortized over 8
matmuls) instead of one per matmul.  Each bank's drain copy is issued right
after that bank's final-tap matmul so drains overlap the remaining banks'
matmuls and the next group never stalls on a busy PSUM bank.

Sharding: data-parallel over batch — 16 images / 8 cores = 2 images per
core, weights replicated, no collectives.
"""

import sys

for _p in ("/opt/trn_rl_repo",):
    if _p not in sys.path:
        sys.path.insert(0, _p)

import numpy as np

import concourse.mybir as mybir
import concourse.tile as tile
from concourse import bacc
from concourse.bass_utils import run_bass_kernel_spmd

B, C, H, W = 16, 128, 192, 192
NCORES = 8
IPC = B // NCORES  # images per core
NTAP = 9
SW = W + 1  # spacer-packed row width (193)
XLEN = 1 + (H + 2) * SW + 4  # leading zero + 194 packed rows + tap margin
RB = 32  # output rows per x-load block
GB = 8  # PSUM banks per matmul group (16 output rows)
MM_DT = mybir.dt.bfloat16
NP_BF16 = mybir.dt.np(mybir.dt.bfloat16)


def _build(ipc, rb, mm_dt, repeat=1, xbufs=3, obufs=2, ahead=1):
    """Emit the per-core Bass program.

    The x-DMA for block k+ahead is issued before block k's compute/out-DMA
    in program order, so input prefetch never queues behind output drains.
    repeat>1 wraps the body in a For_i loop (timing harness only; the body
    is idempotent so outputs are unchanged).
    """
    nc = bacc.Bacc("TRN2", target_bir_lowering=False, debug=False)
    x_in = nc.dram_tensor(
        "x", [ipc, C, XLEN], mm_dt, kind="ExternalInput"
    ).ap()
    w_in = nc.dram_tensor(
        "w", [C, NTAP * C], mm_dt, kind="ExternalInput"
    ).ap()
    y_out = nc.dram_tensor(
        "y", [ipc, C, H, W], mm_dt, kind="ExternalOutput"
    ).ap()

    xtl = (rb + 2) * SW + 4  # x tile flat length per partition
    blocks = [(img, r0) for img in range(ipc) for r0 in range(0, H, rb)]
    rows_per_group = 2 * GB

    with tile.TileContext(nc) as tc:
        with (
            tc.tile_pool(name="wp", bufs=1) as wpool,
            tc.tile_pool(name="xp", bufs=xbufs) as xpool,
            tc.tile_pool(name="op", bufs=obufs) as opool,
            tc.tile_pool(name="ps", bufs=8, space="PSUM") as pspool,
        ):
            wt = wpool.tile([C, NTAP * C], mm_dt)
            nc.sync.dma_start(wt[:], w_in[:])

            def load(img, r0):
                xt = xpool.tile([C, xtl], mm_dt, tag="xt", name=f"xt{img}_{r0}")
                base = r0 * SW
                nc.sync.dma_start(xt[:], x_in[img, :, base : base + xtl])
                return xt

            def body():
                xts = [load(*blocks[k]) for k in range(min(ahead, len(blocks)))]
                for k, (img, r0) in enumerate(blocks):
                    if k + ahead < len(blocks):
                        xts.append(load(*blocks[k + ahead]))
                    xt = xts.pop(0)
                    # output tile keeps the spacer column; the out-DMA below
                    # reads a [.., 0:W] slice to strip it.
                    ot = opool.tile([C, rb, SW], mm_dt, tag="ot",
                                    name=f"ot{img}_{r0}")
                    for g in range(rb // rows_per_group):
                        pss = [
                            pspool.tile([C, 2 * SW], mybir.dt.float32,
                                        tag="ps", name=f"ps{p}")
                            for p in range(GB)
                        ]
                        for t in range(NTAP):
                            kh, kw = divmod(t, 3)
                            nc.tensor.ldweights(wt[:, t * C : (t + 1) * C])
                            for p in range(GB):
                                row = g * rows_per_group + 2 * p
                                off = (row + kh) * SW + kw
                                nc.tensor.matmul(
                                    pss[p][:],
                                    wt[:, t * C : (t + 1) * C],
                                    xt[:, off : off + 2 * SW],
                                    start=(t == 0),
                                    stop=(t == NTAP - 1),
                                )
                                if t == NTAP - 1:
                                    row = g * rows_per_group + 2 * p
                                    eng = (nc.scalar.copy if p % 2 == 0
                                           else nc.vector.tensor_copy)
                                    eng(ot[:, row : row + 2, :].opt(),
                                        pss[p][:])
                    nc.sync.dma_start(
                        y_out[img, :, r0 : r0 + rb, :], ot[:, :, 0:W]
                    )

            if repeat == 1:
                body()
            else:
                with tc.For_i(0, repeat, 1):
                    body()
    nc.compile()
    return nc


def _fold_weights(w_square, w_ver, w_hor, w_diag19, w_diag37):
    """Fold the 5 branches into one 3x3 weight, laid out [C_in, tap*C_out]."""
    eye = np.eye(3, dtype=np.float32)
    anti = eye[::-1, :]
    w_eff = (
        np.asarray(w_square, np.float32)
        + np.asarray(w_diag19, np.float32) * eye
        + np.asarray(w_diag37, np.float32) * anti
    )
    w_eff[:, :, :, 1] += np.asarray(w_ver, np.float32)[:, :, :, 0]
    w_eff[:, :, 1, :] += np.asarray(w_hor, np.float32)[:, :, 0, :]
    # [O, I, KH, KW] -> [I, KH, KW, O] -> [I, (KH*KW)*O]  (lhsT per tap)
    return np.ascontiguousarray(
        w_eff.transpose(1, 2, 3, 0).reshape(C, NTAP * C)
    ).astype(NP_BF16)


def _pack_x(x):
    """[B,C,H,W] -> spacer-packed flat bf16 [B,C,XLEN]."""
    xs = np.zeros((B, C, XLEN), NP_BF16)
    rows = xs[:, :, 1 : 1 + (H + 2) * SW].reshape(B, C, H + 2, SW)
    rows[:, :, 1 : H + 1, 0:W] = np.asarray(x, np.float32).astype(NP_BF16)
    return xs


_nc_cache = {}


def kernel(x, w_square, w_ver, w_hor, w_diag19, w_diag37):
    w_host = _fold_weights(w_square, w_ver, w_hor, w_diag19, w_diag37)
    xs = _pack_x(x)

    if "nc" not in _nc_cache:
        _nc_cache["nc"] = _build(IPC, RB, MM_DT)
    nc = _nc_cache["nc"]

    in_maps = [
        {"x": np.ascontiguousarray(xs[c * IPC : (c + 1) * IPC]), "w": w_host}
        for c in range(NCORES)
    ]
    res = run_bass_kernel_spmd(nc, in_maps, list(range(NCORES)))
    return np.concatenate(
        [res.results[c]["y"] for c in range(NCORES)], axis=0
    ).astype(np.float32)
